# revision 4
# baseline (speedup 1.0000x reference)
"""Trainium2 Bass kernel for nn_Interaction_GraphConvolution (GNN message passing).

Math (N=2048, F_IN=128, F=64):
    H = X @ W + b                                      # [N, F]
    out[j,f] = sum_{i,k} A[j,i] * mh[i,k] * mf[j,k] * H[i,f] * H[k,f]

Sharding: k axis split across 8 cores (256 k's each).

fp8 DoubleRow formulation (per core, k-shard):
    A  = 0.5*J + A',   mh = 0.5*J + mh'          (rank-1 centering)
    R^[i,(k,f)] = e4m3(H[i,f] * mh'[i,k] * H[k,f])     (fp8, scaled by S)
    G  = A'@R^   (fp8 DoubleRow matmuls, 256-deep contraction per MM)
    device out_raw[j,f] = sum_k mf[j,k] * G[j,(k,f)]   (DVE mul + GPSIMD accum)
  Host-side corrections (cheap GEMMs, deterministic):
    out = sum_c out_raw_c/(256*S)
        + 0.5 * sum_c mf_c @ ((mh_s_c.T @ Hbf) * Hbf[shard] / S)   (J@R' term)
        + 0.5 * (A@H) * (mf@H)                                      (mh-mean term)

Device schedule per k-chunk (8 k's = 512 cols, (f,k)-ordered):
    hk   <- partition-broadcast DMA of H shard rows (bf16)
    mhh  = mh'[i,k]*H[k,f]        DVE (2x mode), 16 tiles
    rp   = H[i,f]*mhh -> fp8      DVE, into [128,2,512] DoubleRow pairs
    G    = A'@rp                  8 DoubleRow matmuls per j-tile
    t1   = G * mf[j,k]            DVE (PSUM read)
    acc += t1                     GPSIMD
Final: acc k-reduce (DVE) -> out_raw DMA. No scalar-engine work, no
on-device transposes or casts (A pre-transposed/pre-quantized on host).
"""

import numpy as np
import ml_dtypes

import concourse.bacc as bacc
import concourse.mybir as mybir
from concourse.tile import TileContext
from concourse.bass_utils import run_bass_kernel_spmd

N = 2048
FIN = 128
F = 64
P = 128
NCORES = 8
KSH = N // NCORES          # 256 k's per core
KB = 8                     # k's per chunk (512 matmul cols)
NKB = KSH // KB            # 32 chunks per core
NIT = N // P               # 16 i tiles
NIT2 = NIT // 2            # 8 DoubleRow i-tile pairs
NJT = N // P               # 16 j tiles
NCOL = KB * F              # 512
S = 4.0                    # R' scale (keeps |rp| < 240 for e4m3)

BF16 = ml_dtypes.bfloat16
FP8 = ml_dtypes.float8_e4m3

_CACHE = {}


def _build():
    dt = mybir.dt
    nc = bacc.Bacc("TRN2")

    at8_in = nc.declare_dram_parameter("at8", [N, N], dt.float8e4, isOutput=False)
    mhs_in = nc.declare_dram_parameter("mhs", [N, KSH], dt.bfloat16, isOutput=False)
    mf_in = nc.declare_dram_parameter("mf", [N, KSH], dt.float32, isOutput=False)
    hb_in = nc.declare_dram_parameter("hb", [N, F], dt.bfloat16, isOutput=False)
    hshfk_in = nc.declare_dram_parameter(
        "hshfk", [1, NKB * NCOL], dt.bfloat16, isOutput=False
    )
    out_p = nc.declare_dram_parameter("out_p", [N, F], dt.float32, isOutput=True)

    with TileContext(nc) as tc:
        with (
            tc.tile_pool(name="const", bufs=1) as cpool,
            tc.tile_pool(name="work", bufs=1) as work,
            tc.tile_pool(name="hkp", bufs=3) as hkp,
            tc.tile_pool(name="mhhp", bufs=4) as mhhp,
            tc.tile_pool(name="rpp", bufs=2) as rpp,
            tc.tile_pool(name="t1p", bufs=4) as t1p,
            tc.tile_pool(name="redp", bufs=2) as redp,
            tc.tile_pool(name="psg", bufs=7, space="PSUM") as psg,
        ):
            # ---- resident inputs ----
            atall = cpool.tile([P, NIT2, 2, NJT, P], dt.float8e4)
            for it in range(NIT):
                nc.sync.dma_start(
                    out=atall[:, it // 2, it % 2, :, :],
                    in_=at8_in[it * P:(it + 1) * P, :],
                )
            mhsall = cpool.tile([P, NIT, KSH], dt.bfloat16)
            for it in range(NIT):
                nc.sync.dma_start(
                    out=mhsall[:, it, :], in_=mhs_in[it * P:(it + 1) * P, :]
                )
            mfall = cpool.tile([P, NJT, KSH], dt.float32)
            for jt in range(NJT):
                nc.sync.dma_start(
                    out=mfall[:, jt, :], in_=mf_in[jt * P:(jt + 1) * P, :]
                )
            hball = cpool.tile([P, NIT, F], dt.bfloat16)
            for it in range(NIT):
                nc.sync.dma_start(
                    out=hball[:, it, :], in_=hb_in[it * P:(it + 1) * P, :]
                )

            acc = [work.tile([P, NCOL], dt.float32, tag=f"acc{j}", name=f"acc{j}")
                   for j in range(NJT)]
            for j in range(NJT):
                nc.any.memset(acc[j], 0.0)

            # ---- software-pipelined main loop over k chunks ----
            # DVE builds for chunk kb+1 are interleaved between chunk kb's
            # epilogue ops so the PE never waits at a chunk boundary.
            def load_hk(kb):
                hk = hkp.tile([P, NCOL], dt.bfloat16, tag="hk", name="hk")
                nc.sync.dma_start(
                    out=hk,
                    in_=hshfk_in[0:1, kb * NCOL:(kb + 1) * NCOL]
                    .partition_broadcast(P),
                )
                return hk

            def build_pair(kb, it2, hk):
                """DVE: build rp[it2] (both DoubleRow halves) for chunk kb."""
                hk_v = hk[:, :].rearrange("p (f k) -> p f k", f=F)
                rp_t = rpp.tile([P, 2, NCOL], dt.float8e4, tag=f"rp{it2}",
                                name=f"rp{it2}")
                for d in range(2):
                    it = 2 * it2 + d
                    mhh = mhhp.tile([P, NCOL], dt.bfloat16, tag="mhh",
                                    name="mhh")
                    mhh_v = mhh[:, :].rearrange("p (f k) -> p f k", f=F)
                    mhs_b = (
                        mhsall[:, it, kb * KB:(kb + 1) * KB]
                        .unsqueeze(1)
                        .to_broadcast([P, F, KB])
                    )
                    nc.vector.tensor_mul(mhh_v, mhs_b, hk_v)
                    h_b = (
                        hball[:, it, :]
                        .unsqueeze(2)
                        .to_broadcast([P, F, KB])
                    )
                    rp_v = rp_t[:, d, :].rearrange("p (f k) -> p f k", f=F)
                    nc.vector.tensor_mul(rp_v, h_b, mhh_v)
                return rp_t

            hk_cur = load_hk(0)
            rp_cur = [build_pair(0, it2, hk_cur) for it2 in range(NIT2)]

            for kb in range(NKB):
                hk_nxt = load_hk(kb + 1) if kb + 1 < NKB else None
                rp_nxt = [None] * NIT2
                for jt in range(NJT):
                    if jt < NIT2 and hk_nxt is not None:
                        rp_nxt[jt] = build_pair(kb + 1, jt, hk_nxt)
                    g_ps = psg.tile([P, NCOL], dt.float32, tag="g", name="g")
                    for it2 in range(NIT2):
                        nc.tensor.matmul(
                            g_ps,
                            atall[:, it2, :, jt, :],
                            rp_cur[it2][:, :, :],
                            start=(it2 == 0),
                            stop=(it2 == NIT2 - 1),
                            perf_mode=mybir.MatmulPerfMode.DoubleRow,
                        )
                    t1 = t1p.tile([P, NCOL], dt.float32, tag="t1", name="t1")
                    t1_v = t1[:, :].rearrange("p (f k) -> p f k", f=F)
                    g_v = g_ps[:, :].rearrange("p (f k) -> p f k", f=F)
                    mf_b = (
                        mfall[:, jt, kb * KB:(kb + 1) * KB]
                        .unsqueeze(1)
                        .to_broadcast([P, F, KB])
                    )
                    nc.vector.tensor_mul(t1_v, g_v, mf_b)
                    nc.gpsimd.tensor_add(acc[jt], acc[jt], t1)
                rp_cur = rp_nxt

            # ---- finale: k-reduce, store raw partials ----
            for jt in range(NJT):
                red = redp.tile([P, F], dt.float32, tag="red", name="red")
                nc.vector.tensor_reduce(
                    red,
                    acc[jt][:, :].rearrange("p (f k) -> p f k", f=F),
                    axis=mybir.AxisListType.X,
                    op=mybir.AluOpType.add,
                )
                nc.sync.dma_start(out=out_p[jt * P:(jt + 1) * P, :], in_=red)

    nc.finalize()
    return nc


def _get_nc():
    if "nc" not in _CACHE:
        _CACHE["nc"] = _build()
    return _CACHE["nc"]


def _host_prep(node_features, adjacency_matrix, mask_father, mask_hadamard,
               weight, bias):
    """Quantize/shard inputs; return (in_maps, correction[j,f] fp64)."""
    X = np.asarray(node_features, np.float32)
    A = np.asarray(adjacency_matrix, np.float32)
    mf = np.asarray(mask_father, np.float32)
    mh = np.asarray(mask_hadamard, np.float32)
    W = np.asarray(weight, np.float32)
    b = np.asarray(bias, np.float32)

    H = (X.astype(BF16).astype(np.float32) @ W.astype(BF16).astype(np.float32)
         + b).astype(np.float32)
    Hbf_ml = H.astype(BF16)
    Hbf = Hbf_ml.astype(np.float32)

    aT8 = np.ascontiguousarray((A.T.astype(np.float32) - 0.5) * 256.0).astype(FP8)

    AH = A @ H
    MH = mf @ H
    corr = 0.5 * AH.astype(np.float64) * MH.astype(np.float64)

    maps = []
    for c in range(NCORES):
        sl = slice(c * KSH, (c + 1) * KSH)
        mhs_ml = np.ascontiguousarray(
            S * (mh[:, sl].astype(np.float32) - 0.5)
        ).astype(BF16)
        mf_c = np.ascontiguousarray(mf[:, sl])
        hshfk = np.ascontiguousarray(
            Hbf_ml[sl].reshape(NKB, KB, F).transpose(0, 2, 1)
        ).reshape(1, NKB * NCOL)
        maps.append({
            "at8": aT8,
            "mhs": mhs_ml,
            "mf": mf_c,
            "hb": Hbf_ml,
            "hshfk": hshfk,
        })
        # host-side J@R' correction term for this core's shard
        cs_b = (mhs_ml.astype(np.float32).T @ Hbf) * Hbf[sl] / S   # [KSH, F]
        corr += 0.5 * (mf_c.astype(np.float64) @ cs_b.astype(np.float64))
    return maps, corr


def run_spmd(inputs, **kw):
    """Run the SPMD kernel; returns (summed_output, BassKernelResults)."""
    nc = _get_nc()
    maps, corr = _host_prep(**inputs)
    res = run_bass_kernel_spmd(nc, maps, list(range(NCORES)), **kw)
    out = corr
    for c in range(NCORES):
        out = out + res.results[c]["out_p"].astype(np.float64) / (256.0 * S)
    return out.astype(np.float32), res


def kernel(node_features, adjacency_matrix, mask_father, mask_hadamard,
           weight, bias):
    out, _ = run_spmd(dict(
        node_features=node_features,
        adjacency_matrix=adjacency_matrix,
        mask_father=mask_father,
        mask_hadamard=mask_hadamard,
        weight=weight,
        bias=bias,
    ))
    return out


# revision 5
# speedup vs baseline: 1.0082x; 1.0082x over previous
"""Trainium2 Bass kernel for nn_Interaction_GraphConvolution (GNN message passing).

Math (N=2048, F_IN=128, F=64):
    H = X @ W + b                                      # [N, F]
    out[j,f] = sum_{i,k} A[j,i] * mh[i,k] * mf[j,k] * H[i,f] * H[k,f]

Sharding: k axis split across 8 cores (256 k's each).

fp8 DoubleRow formulation (per core, k-shard):
    A  = 0.5*J + A',   mh = 0.5*J + mh'          (rank-1 centering)
    R^[i,(k,f)] = e4m3(H[i,f] * mh'[i,k] * H[k,f])     (fp8, scaled by S)
    G  = A'@R^   (fp8 DoubleRow matmuls, 256-deep contraction per MM)
    device out_raw[j,f] = sum_k mf[j,k] * G[j,(k,f)]   (DVE mul + GPSIMD accum)
  Host-side corrections (cheap GEMMs, deterministic):
    out = sum_c out_raw_c/(256*S)
        + 0.5 * sum_c mf_c @ ((mh_s_c.T @ Hbf) * Hbf[shard] / S)   (J@R' term)
        + 0.5 * (A@H) * (mf@H)                                      (mh-mean term)

Device schedule per k-chunk (8 k's = 512 cols, (f,k)-ordered):
    hk   <- partition-broadcast DMA of H shard rows (bf16)
    mhh  = mh'[i,k]*H[k,f]        DVE (2x mode), 16 tiles
    rp   = H[i,f]*mhh -> fp8      DVE, into [128,2,512] DoubleRow pairs
    G    = A'@rp                  8 DoubleRow matmuls per j-tile
    t1   = G * mf[j,k]            DVE (PSUM read)
    acc += t1                     GPSIMD
Final: acc k-reduce (DVE) -> out_raw DMA. No scalar-engine work, no
on-device transposes or casts (A pre-transposed/pre-quantized on host).
"""

import numpy as np
import ml_dtypes

import concourse.bacc as bacc
import concourse.mybir as mybir
from concourse.tile import TileContext
from concourse.bass_utils import run_bass_kernel_spmd

N = 2048
FIN = 128
F = 64
P = 128
NCORES = 8
KSH = N // NCORES          # 256 k's per core
KB = 8                     # k's per chunk (512 matmul cols)
NKB = KSH // KB            # 32 chunks per core
NIT = N // P               # 16 i tiles
NIT2 = NIT // 2            # 8 DoubleRow i-tile pairs
NJT = N // P               # 16 j tiles
NCOL = KB * F              # 512
S = 4.0                    # R' scale (keeps |rp| < 240 for e4m3)

BF16 = ml_dtypes.bfloat16
FP8 = ml_dtypes.float8_e4m3

_CACHE = {}


def _build():
    dt = mybir.dt
    nc = bacc.Bacc("TRN2")

    at8_in = nc.declare_dram_parameter("at8", [N, N], dt.float8e4, isOutput=False)
    mhs_in = nc.declare_dram_parameter("mhs", [N, KSH], dt.bfloat16, isOutput=False)
    mf_in = nc.declare_dram_parameter("mf", [N, KSH], dt.float32, isOutput=False)
    hb_in = nc.declare_dram_parameter("hb", [N, F], dt.bfloat16, isOutput=False)
    hshfk_in = nc.declare_dram_parameter(
        "hshfk", [1, NKB * NCOL], dt.bfloat16, isOutput=False
    )
    out_p = nc.declare_dram_parameter("out_p", [N, F], dt.float32, isOutput=True)

    with TileContext(nc) as tc:
        with (
            tc.tile_pool(name="const", bufs=1) as cpool,
            tc.tile_pool(name="work", bufs=1) as work,
            tc.tile_pool(name="hkp", bufs=3) as hkp,
            tc.tile_pool(name="mhhp", bufs=4) as mhhp,
            tc.tile_pool(name="rpp", bufs=2) as rpp,
            tc.tile_pool(name="t1p", bufs=4) as t1p,
            tc.tile_pool(name="redp", bufs=2) as redp,
            tc.tile_pool(name="psg", bufs=7, space="PSUM") as psg,
        ):
            # ---- resident inputs ----
            atall = cpool.tile([P, NIT2, 2, NJT, P], dt.float8e4)
            for it in range(NIT):
                nc.sync.dma_start(
                    out=atall[:, it // 2, it % 2, :, :],
                    in_=at8_in[it * P:(it + 1) * P, :],
                )
            mhsall = cpool.tile([P, NIT, KSH], dt.bfloat16)
            for it in range(NIT):
                nc.sync.dma_start(
                    out=mhsall[:, it, :], in_=mhs_in[it * P:(it + 1) * P, :]
                )
            mfall = cpool.tile([P, NJT, KSH], dt.float32)
            for jt in range(NJT):
                nc.sync.dma_start(
                    out=mfall[:, jt, :], in_=mf_in[jt * P:(jt + 1) * P, :]
                )
            hball = cpool.tile([P, NIT, F], dt.bfloat16)
            for it in range(NIT):
                nc.sync.dma_start(
                    out=hball[:, it, :], in_=hb_in[it * P:(it + 1) * P, :]
                )

            # acc[jt][p, c, (f,k)]: two 512-col halves, one per chunk parity
            acc = [work.tile([P, 2, NCOL], dt.float32, tag=f"acc{j}",
                             name=f"acc{j}") for j in range(NJT)]
            for j in range(NJT):
                nc.any.memset(acc[j], 0.0)

            NPB = NKB // 2      # chunk pairs

            # ---- software-pipelined main loop over chunk PAIRS ----
            # DVE builds for pair pb+1 are interleaved between pair pb's
            # epilogue ops so the PE never waits at a pair boundary.
            def load_hk(pb):
                hk = hkp.tile([P, 2 * NCOL], dt.bfloat16, tag="hk", name="hk")
                nc.sync.dma_start(
                    out=hk,
                    in_=hshfk_in[0:1, pb * 2 * NCOL:(pb + 1) * 2 * NCOL]
                    .partition_broadcast(P),
                )
                return hk

            def build_pair(pb, it2, hk):
                """DVE: build rp[it2] (both DR halves, both chunks) for pair pb."""
                hk_v = hk[:, :].rearrange("p (c f k) -> p c f k", c=2, f=F)
                rp_t = rpp.tile([P, 2, 2 * NCOL], dt.float8e4, tag=f"rp{it2}",
                                name=f"rp{it2}")
                for d in range(2):
                    it = 2 * it2 + d
                    mhh = mhhp.tile([P, 2 * NCOL], dt.bfloat16, tag="mhh",
                                    name="mhh")
                    mhh_v = mhh[:, :].rearrange("p (c f k) -> p c f k",
                                                c=2, f=F)
                    mhs_b = (
                        mhsall[:, it, pb * 2 * KB:(pb + 1) * 2 * KB]
                        .rearrange("p (c k) -> p c k", c=2)
                        .unsqueeze(2)
                        .to_broadcast([P, 2, F, KB])
                    )
                    nc.vector.tensor_mul(mhh_v, mhs_b, hk_v)
                    h_b = (
                        hball[:, it, :]
                        .unsqueeze(1)
                        .unsqueeze(3)
                        .to_broadcast([P, 2, F, KB])
                    )
                    rp_v = rp_t[:, d, :].rearrange("p (c f k) -> p c f k",
                                                   c=2, f=F)
                    nc.vector.tensor_mul(rp_v, h_b, mhh_v)
                return rp_t

            hk_cur = load_hk(0)
            rp_cur = [build_pair(0, it2, hk_cur) for it2 in range(NIT2)]

            for pb in range(NPB):
                hk_nxt = load_hk(pb + 1) if pb + 1 < NPB else None
                rp_nxt = [None] * NIT2
                for jt in range(NJT):
                    if jt < NIT2 and hk_nxt is not None:
                        rp_nxt[jt] = build_pair(pb + 1, jt, hk_nxt)
                    # two psum groups (one per chunk), shared stationaries
                    g_a = psg.tile([P, NCOL], dt.float32, tag="g", name="g")
                    g_b = psg.tile([P, NCOL], dt.float32, tag="g", name="g")
                    for it2 in range(NIT2):
                        for c, g_ps in ((0, g_a), (1, g_b)):
                            nc.tensor.matmul(
                                g_ps,
                                atall[:, it2, :, jt, :],
                                rp_cur[it2][:, :, c * NCOL:(c + 1) * NCOL],
                                start=(it2 == 0),
                                stop=(it2 == NIT2 - 1),
                                perf_mode=mybir.MatmulPerfMode.DoubleRow,
                            )
                    t1 = t1p.tile([P, 2, NCOL], dt.float32, tag="t1",
                                  name="t1")
                    for c, g_ps in ((0, g_a), (1, g_b)):
                        kb = 2 * pb + c
                        t1_v = t1[:, c, :].rearrange("p (f k) -> p f k", f=F)
                        g_v = g_ps[:, :].rearrange("p (f k) -> p f k", f=F)
                        mf_b = (
                            mfall[:, jt, kb * KB:(kb + 1) * KB]
                            .unsqueeze(1)
                            .to_broadcast([P, F, KB])
                        )
                        nc.vector.tensor_mul(t1_v, g_v, mf_b)
                    nc.gpsimd.tensor_add(acc[jt], acc[jt], t1)
                rp_cur = rp_nxt

            # ---- finale: fold chunk-parity halves, k-reduce, store ----
            for jt in range(NJT):
                half = redp.tile([P, NCOL], dt.float32, tag="half",
                                 name="half")
                nc.vector.tensor_add(half, acc[jt][:, 0, :], acc[jt][:, 1, :])
                red = redp.tile([P, F], dt.float32, tag="red", name="red")
                nc.vector.tensor_reduce(
                    red,
                    half[:, :].rearrange("p (f k) -> p f k", f=F),
                    axis=mybir.AxisListType.X,
                    op=mybir.AluOpType.add,
                )
                nc.sync.dma_start(out=out_p[jt * P:(jt + 1) * P, :], in_=red)

    nc.finalize()
    return nc


def _get_nc():
    if "nc" not in _CACHE:
        _CACHE["nc"] = _build()
    return _CACHE["nc"]


def _host_prep(node_features, adjacency_matrix, mask_father, mask_hadamard,
               weight, bias):
    """Quantize/shard inputs; return (in_maps, correction[j,f] fp64)."""
    X = np.asarray(node_features, np.float32)
    A = np.asarray(adjacency_matrix, np.float32)
    mf = np.asarray(mask_father, np.float32)
    mh = np.asarray(mask_hadamard, np.float32)
    W = np.asarray(weight, np.float32)
    b = np.asarray(bias, np.float32)

    H = (X.astype(BF16).astype(np.float32) @ W.astype(BF16).astype(np.float32)
         + b).astype(np.float32)
    Hbf_ml = H.astype(BF16)
    Hbf = Hbf_ml.astype(np.float32)

    aT8 = np.ascontiguousarray((A.T.astype(np.float32) - 0.5) * 256.0).astype(FP8)

    AH = A @ H
    MH = mf @ H
    corr = 0.5 * AH.astype(np.float64) * MH.astype(np.float64)

    maps = []
    for c in range(NCORES):
        sl = slice(c * KSH, (c + 1) * KSH)
        mhs_ml = np.ascontiguousarray(
            S * (mh[:, sl].astype(np.float32) - 0.5)
        ).astype(BF16)
        mf_c = np.ascontiguousarray(mf[:, sl])
        hshfk = np.ascontiguousarray(
            Hbf_ml[sl].reshape(NKB, KB, F).transpose(0, 2, 1)
        ).reshape(1, NKB * NCOL)
        maps.append({
            "at8": aT8,
            "mhs": mhs_ml,
            "mf": mf_c,
            "hb": Hbf_ml,
            "hshfk": hshfk,
        })
        # host-side J@R' correction term for this core's shard
        cs_b = (mhs_ml.astype(np.float32).T @ Hbf) * Hbf[sl] / S   # [KSH, F]
        corr += 0.5 * (mf_c.astype(np.float64) @ cs_b.astype(np.float64))
    return maps, corr


def run_spmd(inputs, **kw):
    """Run the SPMD kernel; returns (summed_output, BassKernelResults)."""
    nc = _get_nc()
    maps, corr = _host_prep(**inputs)
    res = run_bass_kernel_spmd(nc, maps, list(range(NCORES)), **kw)
    out = corr
    for c in range(NCORES):
        out = out + res.results[c]["out_p"].astype(np.float64) / (256.0 * S)
    return out.astype(np.float32), res


def kernel(node_features, adjacency_matrix, mask_father, mask_hadamard,
           weight, bias):
    out, _ = run_spmd(dict(
        node_features=node_features,
        adjacency_matrix=adjacency_matrix,
        mask_father=mask_father,
        mask_hadamard=mask_hadamard,
        weight=weight,
        bias=bias,
    ))
    return out


# revision 10
# speedup vs baseline: 1.0100x; 1.0017x over previous
"""Trainium2 Bass kernel for nn_Interaction_GraphConvolution (GNN message passing).

Math (N=2048, F_IN=128, F=64):
    H = X @ W + b                                      # [N, F]
    out[j,f] = sum_{i,k} A[j,i] * mh[i,k] * mf[j,k] * H[i,f] * H[k,f]

Sharding: k axis split across 8 cores (256 k's each).

fp8 DoubleRow formulation (per core, k-shard):
    A  = 0.5*J + A',   mh = 0.5*J + mh'          (rank-1 centering)
    R^[i,(k,f)] = e4m3(H[i,f] * mh'[i,k] * H[k,f])     (fp8, scaled by S)
    G  = A'@R^   (fp8 DoubleRow matmuls, 256-deep contraction per MM)
    device out_raw[j,f] = sum_k mf[j,k] * G[j,(k,f)]   (DVE mul + GPSIMD accum)
  Host-side corrections (cheap GEMMs, deterministic):
    out = sum_c out_raw_c/(256*S)
        + 0.5 * sum_c mf_c @ ((mh_s_c.T @ Hbf) * Hbf[shard] / S)   (J@R' term)
        + 0.5 * (A@H) * (mf@H)                                      (mh-mean term)

Device schedule per k-chunk (8 k's = 512 cols, (f,k)-ordered):
    hk   <- partition-broadcast DMA of H shard rows (bf16)
    mhh  = mh'[i,k]*H[k,f]        DVE (2x mode), 16 tiles
    rp   = H[i,f]*mhh -> fp8      DVE, into [128,2,512] DoubleRow pairs
    G    = A'@rp                  8 DoubleRow matmuls per j-tile
    t1   = G * mf[j,k]            DVE (PSUM read)
    acc += t1                     GPSIMD
Final: acc k-reduce (DVE) -> out_raw DMA. No scalar-engine work, no
on-device transposes or casts (A pre-transposed/pre-quantized on host).
"""

import numpy as np
import ml_dtypes

import concourse.bacc as bacc
import concourse.mybir as mybir
from concourse.tile import TileContext
from concourse.bass_utils import run_bass_kernel_spmd

N = 2048
FIN = 128
F = 64
P = 128
NCORES = 8
KSH = N // NCORES          # 256 k's per core
KB = 8                     # k's per chunk (512 matmul cols)
NKB = KSH // KB            # 32 chunks per core
NIT = N // P               # 16 i tiles
NIT2 = NIT // 2            # 8 DoubleRow i-tile pairs
NJT = N // P               # 16 j tiles
NCOL = KB * F              # 512
S = 4.0                    # R' scale (keeps |rp| < 240 for e4m3)

BF16 = ml_dtypes.bfloat16
FP8 = ml_dtypes.float8_e4m3

_CACHE = {}


def _build():
    dt = mybir.dt
    nc = bacc.Bacc("TRN2")

    at8_in = nc.declare_dram_parameter("at8", [N, N], dt.float8e4, isOutput=False)
    mhs_in = nc.declare_dram_parameter("mhs", [N, KSH], dt.bfloat16, isOutput=False)
    mf_in = nc.declare_dram_parameter("mf", [N, KSH], dt.float32, isOutput=False)
    hb_in = nc.declare_dram_parameter("hb", [N, F], dt.bfloat16, isOutput=False)
    hshfk_in = nc.declare_dram_parameter(
        "hshfk", [1, NKB * NCOL], dt.bfloat16, isOutput=False
    )
    out_p = nc.declare_dram_parameter("out_p", [N, F], dt.float32, isOutput=True)

    with TileContext(nc) as tc:
        with (
            tc.tile_pool(name="const", bufs=1) as cpool,
            tc.tile_pool(name="work", bufs=1) as work,
            tc.tile_pool(name="hkp", bufs=2) as hkp,
            tc.tile_pool(name="mhhp", bufs=3) as mhhp,
            tc.tile_pool(name="rpp", bufs=2) as rpp,
            tc.tile_pool(name="t1p", bufs=6) as t1p,
            tc.tile_pool(name="redp", bufs=4) as redp,
            tc.tile_pool(name="psg", bufs=7, space="PSUM") as psg,
        ):
            # ---- resident inputs ----
            atall = cpool.tile([P, NIT2, 2, NJT, P], dt.float8e4)
            for it in range(NIT):
                nc.sync.dma_start(
                    out=atall[:, it // 2, it % 2, :, :],
                    in_=at8_in[it * P:(it + 1) * P, :],
                )
            mhsall = cpool.tile([P, NIT, KSH], dt.bfloat16)
            for it in range(NIT):
                nc.sync.dma_start(
                    out=mhsall[:, it, :], in_=mhs_in[it * P:(it + 1) * P, :]
                )
            mfall = cpool.tile([P, NJT, KSH], dt.float32)
            for jt in range(NJT):
                nc.sync.dma_start(
                    out=mfall[:, jt, :], in_=mf_in[jt * P:(jt + 1) * P, :]
                )
            hball = cpool.tile([P, NIT, F], dt.bfloat16)
            for it in range(NIT):
                nc.sync.dma_start(
                    out=hball[:, it, :], in_=hb_in[it * P:(it + 1) * P, :]
                )

            # acc[jt][p, c, (f,k)]: two 512-col halves, one per chunk parity
            acc = [work.tile([P, 2, NCOL], dt.float32, tag=f"acc{j}",
                             name=f"acc{j}") for j in range(NJT)]
            for j in range(NJT):
                nc.any.memset(acc[j], 0.0)

            NPB = NKB // 2      # chunk pairs

            # ---- software-pipelined main loop over chunk PAIRS ----
            # DVE builds for pair pb+1 are interleaved between pair pb's
            # epilogue ops so the PE never waits at a pair boundary.
            def load_hk(pb):
                hk = hkp.tile([P, 2 * NCOL], dt.bfloat16, tag="hk", name="hk")
                nc.sync.dma_start(
                    out=hk,
                    in_=hshfk_in[0:1, pb * 2 * NCOL:(pb + 1) * 2 * NCOL]
                    .partition_broadcast(P),
                )
                return hk

            def build_ops(pb, hk):
                """Yield DVE build closures for pair pb: (mhh, rp) per
                DR-half; 32 ops total, rp tiles returned up front."""
                hk_v = hk[:, :].rearrange("p (c f k) -> p c f k", c=2, f=F)
                rp_ts = [rpp.tile([P, 2, 2 * NCOL], dt.float8e4,
                                  tag=f"rp{i}", name=f"rp{i}")
                         for i in range(NIT2)]

                def make(it2, d):
                    def emit():
                        it = 2 * it2 + d
                        mhh = mhhp.tile([P, 2 * NCOL], dt.bfloat16,
                                        tag="mhh", name="mhh")
                        mhh_v = mhh[:, :].rearrange(
                            "p (c f k) -> p c f k", c=2, f=F)
                        mhs_b = (
                            mhsall[:, it, pb * 2 * KB:(pb + 1) * 2 * KB]
                            .rearrange("p (c k) -> p c k", c=2)
                            .unsqueeze(2)
                            .to_broadcast([P, 2, F, KB])
                        )
                        nc.vector.tensor_mul(mhh_v, mhs_b, hk_v)
                        h_b = (
                            hball[:, it, :]
                            .unsqueeze(1)
                            .unsqueeze(3)
                            .to_broadcast([P, 2, F, KB])
                        )
                        rp_v = rp_ts[it2][:, d, :].rearrange(
                            "p (c f k) -> p c f k", c=2, f=F)
                        nc.vector.tensor_mul(rp_v, h_b, mhh_v)
                    return emit

                ops = [make(it2, d) for it2 in range(NIT2) for d in range(2)]
                return rp_ts, ops

            hk_cur = load_hk(0)
            rp_cur, ops0 = build_ops(0, hk_cur)
            for op in ops0:
                op()

            for pb in range(NPB):
                if pb + 1 < NPB:
                    hk_nxt = load_hk(pb + 1)
                    rp_nxt, bops = build_ops(pb + 1, hk_nxt)
                else:
                    rp_nxt, bops = None, []
                for jt in range(NJT):
                    # two psum groups (one per chunk), shared stationaries
                    g_a = psg.tile([P, NCOL], dt.float32, tag="g", name="g")
                    g_b = psg.tile([P, NCOL], dt.float32, tag="g", name="g")
                    for it2 in range(NIT2):
                        for c, g_ps in ((0, g_a), (1, g_b)):
                            nc.tensor.matmul(
                                g_ps,
                                atall[:, it2, :, jt, :],
                                rp_cur[it2][:, :, c * NCOL:(c + 1) * NCOL],
                                start=(it2 == 0),
                                stop=(it2 == NIT2 - 1),
                                perf_mode=mybir.MatmulPerfMode.DoubleRow,
                            )
                    # epilogue: t1 = G*mf (DVE, drains PSUM), then gpsimd
                    # fold+reduce+accumulate into acc[jt][P,F]
                    t1 = t1p.tile([P, 2, NCOL], dt.float32, tag="t1",
                                  name="t1")
                    for c, g_ps in ((0, g_a), (1, g_b)):
                        kb = 2 * pb + c
                        t1_v = t1[:, c, :].rearrange("p (f k) -> p f k", f=F)
                        g_v = g_ps[:, :].rearrange("p (f k) -> p f k", f=F)
                        mf_b = (
                            mfall[:, jt, kb * KB:(kb + 1) * KB]
                            .unsqueeze(1)
                            .to_broadcast([P, F, KB])
                        )
                        nc.vector.tensor_mul(t1_v, g_v, mf_b)
                    nc.gpsimd.tensor_add(acc[jt], acc[jt], t1)
                    # interleave next pair's DVE builds behind the t1s
                    for op in bops[2 * jt:2 * jt + 2]:
                        op()
                rp_cur = rp_nxt

            # ---- finale: fold chunk-parity halves, k-reduce, store ----
            for jt in range(NJT):
                half = redp.tile([P, NCOL], dt.float32, tag="half",
                                 name="half")
                nc.vector.tensor_add(half, acc[jt][:, 0, :], acc[jt][:, 1, :])
                red = redp.tile([P, F], dt.float32, tag="red", name="red")
                nc.vector.tensor_reduce(
                    red,
                    half[:, :].rearrange("p (f k) -> p f k", f=F),
                    axis=mybir.AxisListType.X,
                    op=mybir.AluOpType.add,
                )
                nc.sync.dma_start(out=out_p[jt * P:(jt + 1) * P, :], in_=red)

    nc.finalize()
    return nc


def _get_nc():
    if "nc" not in _CACHE:
        _CACHE["nc"] = _build()
    return _CACHE["nc"]


def _host_prep(node_features, adjacency_matrix, mask_father, mask_hadamard,
               weight, bias):
    """Quantize/shard inputs; return (in_maps, correction[j,f] fp64)."""
    X = np.asarray(node_features, np.float32)
    A = np.asarray(adjacency_matrix, np.float32)
    mf = np.asarray(mask_father, np.float32)
    mh = np.asarray(mask_hadamard, np.float32)
    W = np.asarray(weight, np.float32)
    b = np.asarray(bias, np.float32)

    H = (X.astype(BF16).astype(np.float32) @ W.astype(BF16).astype(np.float32)
         + b).astype(np.float32)
    Hbf_ml = H.astype(BF16)
    Hbf = Hbf_ml.astype(np.float32)

    aT8 = np.ascontiguousarray((A.T.astype(np.float32) - 0.5) * 256.0).astype(FP8)

    AH = A @ H
    MH = mf @ H
    corr = 0.5 * AH.astype(np.float64) * MH.astype(np.float64)

    maps = []
    for c in range(NCORES):
        sl = slice(c * KSH, (c + 1) * KSH)
        mhs_ml = np.ascontiguousarray(
            S * (mh[:, sl].astype(np.float32) - 0.5)
        ).astype(BF16)
        mf_c = np.ascontiguousarray(mf[:, sl])
        hshfk = np.ascontiguousarray(
            Hbf_ml[sl].reshape(NKB, KB, F).transpose(0, 2, 1)
        ).reshape(1, NKB * NCOL)
        maps.append({
            "at8": aT8,
            "mhs": mhs_ml,
            "mf": mf_c,
            "hb": Hbf_ml,
            "hshfk": hshfk,
        })
        # host-side J@R' correction term for this core's shard
        cs_b = (mhs_ml.astype(np.float32).T @ Hbf) * Hbf[sl] / S   # [KSH, F]
        corr += 0.5 * (mf_c.astype(np.float64) @ cs_b.astype(np.float64))
    return maps, corr


def run_spmd(inputs, **kw):
    """Run the SPMD kernel; returns (summed_output, BassKernelResults)."""
    nc = _get_nc()
    maps, corr = _host_prep(**inputs)
    res = run_bass_kernel_spmd(nc, maps, list(range(NCORES)), **kw)
    out = corr
    for c in range(NCORES):
        out = out + res.results[c]["out_p"].astype(np.float64) / (256.0 * S)
    return out.astype(np.float32), res


def kernel(node_features, adjacency_matrix, mask_father, mask_hadamard,
           weight, bias):
    out, _ = run_spmd(dict(
        node_features=node_features,
        adjacency_matrix=adjacency_matrix,
        mask_father=mask_father,
        mask_hadamard=mask_hadamard,
        weight=weight,
        bias=bias,
    ))
    return out


# revision 13
# speedup vs baseline: 1.3431x; 1.3299x over previous
"""Trainium2 Bass kernel for nn_Interaction_GraphConvolution (GNN message passing).

Math (N=2048, F_IN=128, F=64):
    H = X @ W + b                                      # [N, F]
    out[j,f] = sum_{i,k} A[j,i] * mh[i,k] * mf[j,k] * H[i,f] * H[k,f]

Sharding: k axis split across 8 cores (256 k's each).

fp8 DoubleRow formulation (per core, k-shard):
    A  = 0.5*J + A',   mh = 0.5*J + mh'          (rank-1 centering)
    R^[i,(k,f)] = e4m3(H[i,f] * mh'[i,k] * H[k,f])     (fp8, scaled by S)
    G  = A'@R^   (fp8 DoubleRow matmuls, 256-deep contraction per MM)
    device out_raw[j,f] = sum_k mf[j,k] * G[j,(k,f)]   (DVE mul + GPSIMD accum)
  Host-side corrections (cheap GEMMs, deterministic):
    out = sum_c out_raw_c/(256*S)
        + 0.5 * sum_c mf_c @ ((mh_s_c.T @ Hbf) * Hbf[shard] / S)   (J@R' term)
        + 0.5 * (A@H) * (mf@H)                                      (mh-mean term)

Device schedule per k-chunk (8 k's = 512 cols, (f,k)-ordered):
    hk   <- partition-broadcast DMA of H shard rows (bf16)
    mhh  = mh'[i,k]*H[k,f]        DVE (2x mode), 16 tiles
    rp   = H[i,f]*mhh -> fp8      DVE, into [128,2,512] DoubleRow pairs
    G    = A'@rp                  8 DoubleRow matmuls per j-tile
    t1   = G * mf[j,k]            DVE (PSUM read)
    acc += t1                     GPSIMD
Final: acc k-reduce (DVE) -> out_raw DMA. No scalar-engine work, no
on-device transposes or casts (A pre-transposed/pre-quantized on host).
"""

import numpy as np
import ml_dtypes

import concourse.bacc as bacc
import concourse.mybir as mybir
from concourse.tile import TileContext
from concourse.bass_utils import run_bass_kernel_spmd

N = 2048
FIN = 128
F = 64
P = 128
NCORES = 8
KSH = N // NCORES          # 256 k's per core
KB = 8                     # k's per chunk (512 matmul cols)
NKB = KSH // KB            # 32 chunks per core
NIT = N // P               # 16 i tiles
NIT2 = NIT // 2            # 8 DoubleRow i-tile pairs
NJT = N // P               # 16 j tiles
NCOL = KB * F              # 512
S = 4.0                    # R' scale (keeps |rp| < 240 for e4m3)

BF16 = ml_dtypes.bfloat16
FP8 = ml_dtypes.float8_e4m3

_CACHE = {}


def _build():
    dt = mybir.dt
    nc = bacc.Bacc("TRN2")

    at8_in = nc.declare_dram_parameter("at8", [N, N], dt.float8e4, isOutput=False)
    mhs_in = nc.declare_dram_parameter("mhs", [N, KSH], dt.bfloat16, isOutput=False)
    mf_in = nc.declare_dram_parameter("mf", [N, KSH], dt.float32, isOutput=False)
    hb_in = nc.declare_dram_parameter("hb", [N, F], dt.bfloat16, isOutput=False)
    hshfk_in = nc.declare_dram_parameter(
        "hshfk", [1, NKB * NCOL], dt.bfloat16, isOutput=False
    )
    out_p = nc.declare_dram_parameter("out_p", [N, F], dt.float32, isOutput=True)

    with TileContext(nc) as tc:
        with (
            tc.tile_pool(name="const", bufs=1) as cpool,
            tc.tile_pool(name="work", bufs=1) as work,
            tc.tile_pool(name="hkp", bufs=2) as hkp,
            tc.tile_pool(name="mhhp", bufs=2) as mhhp,
            tc.tile_pool(name="rpp", bufs=2) as rpp,
            tc.tile_pool(name="t1p", bufs=8) as t1p,
            tc.tile_pool(name="rbp", bufs=4) as rbp,
            tc.tile_pool(name="redp", bufs=4) as redp,
            tc.tile_pool(name="psg", bufs=8, space="PSUM") as psg,
        ):
            # ---- resident inputs ----
            atall = cpool.tile([P, NIT2, 2, NJT, P], dt.float8e4)
            for it in range(NIT):
                nc.sync.dma_start(
                    out=atall[:, it // 2, it % 2, :, :],
                    in_=at8_in[it * P:(it + 1) * P, :],
                )
            mhsall = cpool.tile([P, NIT, KSH], dt.bfloat16)
            for it in range(NIT):
                nc.sync.dma_start(
                    out=mhsall[:, it, :], in_=mhs_in[it * P:(it + 1) * P, :]
                )
            mfall = cpool.tile([P, NJT, KSH], dt.float32)
            for jt in range(NJT):
                nc.sync.dma_start(
                    out=mfall[:, jt, :], in_=mf_in[jt * P:(jt + 1) * P, :]
                )
            hball = cpool.tile([P, NIT, F], dt.bfloat16)
            for it in range(NIT):
                nc.sync.dma_start(
                    out=hball[:, it, :], in_=hb_in[it * P:(it + 1) * P, :]
                )
            # hx[it][p, (f,k)] = H[i,f] replicated over k (packed operand
            # so the rp build runs in DVE 2x mode)
            hxall = cpool.tile([P, NIT, NCOL], dt.bfloat16)
            for it in range(NIT):
                nc.vector.tensor_copy(
                    out=hxall[:, it, :].rearrange("p (f k) -> p f k", f=F),
                    in_=hball[:, it, :].unsqueeze(2).to_broadcast([P, F, KB]),
                )

            # acc[jt][p, (f,k)]: running sum over all chunks (both parities
            # share k-slots; epilogue adds each chunk separately)
            acc = [work.tile([P, NCOL], dt.float32, tag=f"acc{j}",
                             name=f"acc{j}") for j in range(NJT)]
            for j in range(NJT):
                nc.any.memset(acc[j], 0.0)

            NPB = NKB // 2      # chunk pairs

            # ---- software-pipelined main loop over chunk PAIRS ----
            # DVE builds for pair pb+1 are interleaved between pair pb's
            # epilogue ops so the PE never waits at a pair boundary.
            def load_hk(pb):
                hk = hkp.tile([P, 2 * NCOL], dt.bfloat16, tag="hk", name="hk")
                nc.sync.dma_start(
                    out=hk,
                    in_=hshfk_in[0:1, pb * 2 * NCOL:(pb + 1) * 2 * NCOL]
                    .partition_broadcast(P),
                )
                return hk

            def build_ops(pb, hk):
                """Build closures for pair pb: 8 batched mhh ops (DVE 2x),
                16 rpb ops (DVE 2x, bf16), 16 fp8 casts (Scalar)."""
                rp_ts = [rpp.tile([P, 2, 2 * NCOL], dt.float8e4,
                                  tag=f"rp{i}", name=f"rp{i}")
                         for i in range(NIT2)]
                blk_holder = {}

                def make_mhh(b, c):
                    def emit():
                        # blk[p, i4, f, k] = mh'[i,k] * H[k,f]  (chunk c)
                        blk = mhhp.tile([P, 4, F, KB], dt.bfloat16,
                                        tag="mhhblk", name="mhhblk")
                        kb = 2 * pb + c
                        mhs_b = (
                            mhsall[:, 4 * b:4 * b + 4,
                                   kb * KB:(kb + 1) * KB]
                            .unsqueeze(2)
                            .to_broadcast([P, 4, F, KB])
                        )
                        hk_b = (
                            hk[:, c * NCOL:(c + 1) * NCOL]
                            .rearrange("p (f k) -> p f k", f=F)
                            .unsqueeze(1)
                            .to_broadcast([P, 4, F, KB])
                        )
                        nc.vector.tensor_mul(blk, mhs_b, hk_b)
                        blk_holder[(b, c)] = blk
                    return emit

                def make_rp(it2, c):
                    def emit():
                        b, off = it2 // 2, (it2 % 2) * 2
                        blk = blk_holder[(b, c)]
                        # rpb[p, d, (f,k)] = H[i,f] * mhh   (i = 2*it2+d)
                        rpb = rbp.tile([P, 2, NCOL], dt.bfloat16,
                                       tag="rpb", name="rpb")
                        nc.vector.tensor_mul(
                            rpb,
                            hxall[:, 2 * it2:2 * it2 + 2, :],
                            blk[:, off:off + 2, :, :].rearrange(
                                "p d f k -> p d (f k)"),
                        )
                        # fp8 quantize on the (idle) scalar engine
                        nc.scalar.copy(
                            out=rp_ts[it2][:, :, c * NCOL:(c + 1) * NCOL],
                            in_=rpb,
                        )
                    return emit

                ops = []
                for b in range(4):
                    for c in range(2):
                        ops.append(make_mhh(b, c))
                        ops.append(make_rp(2 * b, c))
                        ops.append(make_rp(2 * b + 1, c))
                return rp_ts, ops

            hk_cur = load_hk(0)
            rp_cur, ops0 = build_ops(0, hk_cur)
            for op in ops0:
                op()

            for pb in range(NPB):
                if pb + 1 < NPB:
                    hk_nxt = load_hk(pb + 1)
                    rp_nxt, bops = build_ops(pb + 1, hk_nxt)
                else:
                    rp_nxt, bops = None, []
                for jt in range(NJT):
                    # two psum groups (one per chunk), shared stationaries
                    g_a = psg.tile([P, NCOL], dt.float32, tag="g", name="g")
                    g_b = psg.tile([P, NCOL], dt.float32, tag="g", name="g")
                    for it2 in range(NIT2):
                        for c, g_ps in ((0, g_a), (1, g_b)):
                            nc.tensor.matmul(
                                g_ps,
                                atall[:, it2, :, jt, :],
                                rp_cur[it2][:, :, c * NCOL:(c + 1) * NCOL],
                                start=(it2 == 0),
                                stop=(it2 == NIT2 - 1),
                                perf_mode=mybir.MatmulPerfMode.DoubleRow,
                            )
                    # epilogue: t1 = G*mf (DVE, drains PSUM), then gpsimd
                    # fold+reduce+accumulate into acc[jt][P,F]
                    for c, g_ps in ((0, g_a), (1, g_b)):
                        kb = 2 * pb + c
                        t1 = t1p.tile([P, NCOL], dt.float32, tag="t1",
                                      name="t1")
                        t1_v = t1[:, :].rearrange("p (f k) -> p f k", f=F)
                        g_v = g_ps[:, :].rearrange("p (f k) -> p f k", f=F)
                        mf_b = (
                            mfall[:, jt, kb * KB:(kb + 1) * KB]
                            .unsqueeze(1)
                            .to_broadcast([P, F, KB])
                        )
                        nc.vector.tensor_mul(t1_v, g_v, mf_b)
                        nc.gpsimd.tensor_add(acc[jt], acc[jt], t1)
                    # interleave next pair's builds behind the t1s
                    for op in bops[(24 * jt) // NJT:(24 * (jt + 1)) // NJT]:
                        op()
                rp_cur = rp_nxt

            # ---- finale: k-reduce, store ----
            for jt in range(NJT):
                red = redp.tile([P, F], dt.float32, tag="red", name="red")
                nc.vector.tensor_reduce(
                    red,
                    acc[jt][:, :].rearrange("p (f k) -> p f k", f=F),
                    axis=mybir.AxisListType.X,
                    op=mybir.AluOpType.add,
                )
                nc.sync.dma_start(out=out_p[jt * P:(jt + 1) * P, :], in_=red)

    nc.finalize()
    return nc


def _get_nc():
    if "nc" not in _CACHE:
        _CACHE["nc"] = _build()
    return _CACHE["nc"]


def _host_prep(node_features, adjacency_matrix, mask_father, mask_hadamard,
               weight, bias):
    """Quantize/shard inputs; return (in_maps, correction[j,f] fp64)."""
    X = np.asarray(node_features, np.float32)
    A = np.asarray(adjacency_matrix, np.float32)
    mf = np.asarray(mask_father, np.float32)
    mh = np.asarray(mask_hadamard, np.float32)
    W = np.asarray(weight, np.float32)
    b = np.asarray(bias, np.float32)

    H = (X.astype(BF16).astype(np.float32) @ W.astype(BF16).astype(np.float32)
         + b).astype(np.float32)
    Hbf_ml = H.astype(BF16)
    Hbf = Hbf_ml.astype(np.float32)

    aT8 = np.ascontiguousarray((A.T.astype(np.float32) - 0.5) * 256.0).astype(FP8)

    AH = A @ H
    MH = mf @ H
    corr = 0.5 * AH.astype(np.float64) * MH.astype(np.float64)

    maps = []
    for c in range(NCORES):
        sl = slice(c * KSH, (c + 1) * KSH)
        mhs_ml = np.ascontiguousarray(
            S * (mh[:, sl].astype(np.float32) - 0.5)
        ).astype(BF16)
        mf_c = np.ascontiguousarray(mf[:, sl])
        hshfk = np.ascontiguousarray(
            Hbf_ml[sl].reshape(NKB, KB, F).transpose(0, 2, 1)
        ).reshape(1, NKB * NCOL)
        maps.append({
            "at8": aT8,
            "mhs": mhs_ml,
            "mf": mf_c,
            "hb": Hbf_ml,
            "hshfk": hshfk,
        })
        # host-side J@R' correction term for this core's shard
        cs_b = (mhs_ml.astype(np.float32).T @ Hbf) * Hbf[sl] / S   # [KSH, F]
        corr += 0.5 * (mf_c.astype(np.float64) @ cs_b.astype(np.float64))
    return maps, corr


def run_spmd(inputs, **kw):
    """Run the SPMD kernel; returns (summed_output, BassKernelResults)."""
    nc = _get_nc()
    maps, corr = _host_prep(**inputs)
    res = run_bass_kernel_spmd(nc, maps, list(range(NCORES)), **kw)
    out = corr
    for c in range(NCORES):
        out = out + res.results[c]["out_p"].astype(np.float64) / (256.0 * S)
    return out.astype(np.float32), res


def kernel(node_features, adjacency_matrix, mask_father, mask_hadamard,
           weight, bias):
    out, _ = run_spmd(dict(
        node_features=node_features,
        adjacency_matrix=adjacency_matrix,
        mask_father=mask_father,
        mask_hadamard=mask_hadamard,
        weight=weight,
        bias=bias,
    ))
    return out
